# revision 10
# baseline (speedup 1.0000x reference)
"""Trainium2 kernel for nn_KeyedLayer: out = (W_sparse @ x.T).T

W is [16384, 16384] sparse COO (rows sorted, ~128 nnz/row, 2M nnz),
x is [64, 16384] fp32.  Strategy: shard output rows across 8 cores
(2048 rows each; disjoint outputs, no collectives).  Each core computes
out.T[2048, 64] = W_core @ x.T as a dense matmul with W densified on
the host in fp8 e3m4 (1.3e-2 rel err, well within tolerance) as the
stationary operand and x fp16 as the moving operand; K=16384 contracted
in 128 blocks through PSUM.

The kernel is DMA-bound: 32 MiB of fp8 W per core at ~360 B/ns (~93 us)
plus 2 MiB x.  W streams in two row-half passes (rows 0-1023 across all
k, then 1024-2047) so the first half's PSUM copyback and store are
hidden under the second pass; the final chunks are single k-blocks so
the tail after the last byte is short.
"""

import os
from contextlib import ExitStack

import numpy as np
import ml_dtypes

import concourse.bass as bass
import concourse.tile as tile
from concourse import bacc, mybir
from concourse.bass_utils import run_bass_kernel_spmd

B = 64
IN_DIM = 16384
OUT_DIM = 16384
N_CORES = 8
ROWS_PER_CORE = OUT_DIM // N_CORES  # 2048
KBLK = IN_DIM // 128  # 128 k-blocks of 128
NT = ROWS_PER_CORE // 128  # 16 row-tiles of 128 rows
NH = NT // 2  # row-tiles per half (8 = one PSUM bank)
RH = ROWS_PER_CORE // 2  # rows per half (1024)

# Each half-pass streams KBLK k-blocks of [128, 1024] fp8 (128 KiB each):
# KGRP k-blocks per DMA chunk for the bulk, single-k-block tail chunks.
KGRP = int(os.environ.get("KERNEL_KGRP", "16"))
NTAIL = int(os.environ.get("KERNEL_NTAIL", "16"))  # trailing 1-kblock chunks
NBULK = (KBLK - NTAIL) // KGRP
assert NBULK * KGRP + NTAIL == KBLK
WBUFS = int(os.environ.get("KERNEL_WBUFS", "6"))

F16 = mybir.dt.float16
FP8 = mybir.dt.float8e3  # e3m4
FP8_NP = ml_dtypes.float8_e3m4
F32 = mybir.dt.float32

_CACHE = {}

LAST_RESULT = None  # BassKernelResults of the most recent run (for test.py)


def _build_program():
    if "nc" in _CACHE:
        return _CACHE["nc"]
    nc = bacc.Bacc(
        "TRN2", target_bir_lowering=False, debug=False, num_devices=N_CORES
    )
    xT_d = nc.dram_tensor("xT", [128, KBLK, B], F16, kind="ExternalInput")
    # wt[h, g, p, j*1024 + r] = W[base + h*1024 + r, k*128 + p],
    # k = g*KGRP + j   (bulk chunks of half-pass h)
    wt_d = nc.dram_tensor("wt", [2, NBULK, 128, KGRP * RH], FP8,
                          kind="ExternalInput")
    if NTAIL:
        wtl_d = nc.dram_tensor("wtl", [2, NTAIL, 128, RH], FP8,
                               kind="ExternalInput")
    # out[p, t, b] = out[b, base + t*128 + p] (host untangles the layout)
    out_d = nc.dram_tensor("out", [128, NT, B], F16, kind="ExternalOutput")

    with tile.TileContext(nc) as tc, ExitStack() as ctx:
        xpool = ctx.enter_context(tc.tile_pool(name="x", bufs=1))
        wpool = ctx.enter_context(tc.tile_pool(name="w", bufs=WBUFS))
        opool = ctx.enter_context(tc.tile_pool(name="o", bufs=1))
        pspool = ctx.enter_context(
            tc.tile_pool(name="ps", bufs=1, space=bass.MemorySpace.PSUM)
        )

        xsb = xpool.tile([128, KBLK, B], F16)  # 2 MiB
        nc.sync.dma_start(xsb[:], xT_d[:])

        # out.T as 16 tiles of [128 rows, 64 batch] fp32 = 4 KiB/partition;
        # half h lives in its own PSUM bank.
        psum = pspool.tile([128, NT, B], F32)
        osb = opool.tile([128, NT, B], F16)

        for h in range(2):
            def kblock_matmuls(wap, k):
                # wap: [128 (k-part), NH*128 rows] fp8 for this k-block
                for i in range(NH):
                    t = h * NH + i
                    nc.tensor.matmul(
                        psum[:, t, :],
                        wap[:, i * 128:(i + 1) * 128],  # lhsT [128, 128] fp8
                        xsb[:, k, :],                   # rhs  [128, 64] fp16
                        # PSUM "start" zeroes the whole 2 KiB bank (zero
                        # region): exactly one start/stop per bank.
                        start=(k == 0 and i == 0),
                        stop=(k == KBLK - 1 and i == NH - 1),
                        skip_group_check=True,
                    )

            for g in range(NBULK):
                wsb = wpool.tile([128, KGRP, RH], FP8)  # 2 MiB at KGRP=16
                nc.sync.dma_start(wsb[:], wt_d[h, g])
                for j in range(KGRP):
                    kblock_matmuls(wsb[:, j, :], g * KGRP + j)
            for i in range(NTAIL):
                wsb = wpool.tile([128, 1, RH], FP8)  # 128 KiB
                nc.sync.dma_start(wsb[:, 0, :], wtl_d[h, i])
                kblock_matmuls(wsb[:, 0, :], NBULK * KGRP + i)

            # PSUM -> SBUF fp16 copyback for this half on DVE, then store
            # via a DVE-issued DMA (skips the SP dispatch hop).  For h=0
            # this hides entirely under the h=1 W stream.
            sl = slice(h * NH, (h + 1) * NH)
            nc.vector.tensor_copy(osb[:, sl, :], psum[:, sl, :])
            nc.scalar.dma_start(out_d.ap()[:, sl, :], osb[:, sl, :])

    nc.compile()
    _CACHE["nc"] = nc
    return nc


def kernel(x_affine: np.ndarray, rows: np.ndarray, cols: np.ndarray,
           vals: np.ndarray) -> np.ndarray:
    global LAST_RESULT
    import scipy.sparse as sp

    x_affine = np.asarray(x_affine, dtype=np.float32)
    rows = np.asarray(rows, dtype=np.int64)
    cols = np.asarray(cols, dtype=np.int64)
    vals = np.asarray(vals, dtype=np.float32)

    # xT host layout [p, k, b]: element = x[b, k*128 + p]
    xT = np.ascontiguousarray(
        x_affine.T.reshape(KBLK, 128, B).transpose(1, 0, 2)
    ).astype(np.float16)

    # rows is sorted; slice each core's nnz range and densify only its
    # [16384, 2048] W.T block (duplicates are summed by scipy).
    in_maps = []
    for c in range(N_CORES):
        base = c * ROWS_PER_CORE
        lo, hi = np.searchsorted(rows, [base, base + ROWS_PER_CORE])
        w_slice = sp.coo_matrix(
            (vals[lo:hi], (cols[lo:hi], rows[lo:hi] - base)),
            shape=(IN_DIM, ROWS_PER_CORE),
        ).toarray()  # [16384, 2048] fp32, w_slice[k, r] = W[base+r, k]
        # wf[k, p, h, r] = W[base + h*1024 + r, k*128 + p]
        wf = w_slice.astype(FP8_NP).reshape(KBLK, 128, 2, RH)
        nb = NBULK * KGRP
        # -> [h, kb, p, r]
        whf = wf.transpose(2, 0, 1, 3)
        wt = np.ascontiguousarray(
            whf[:, :nb].reshape(2, NBULK, KGRP, 128, RH)
            .transpose(0, 1, 3, 2, 4)
        ).reshape(2, NBULK, 128, KGRP * RH)
        m = {"xT": xT, "wt": wt}
        if NTAIL:
            m["wtl"] = np.ascontiguousarray(whf[:, nb:])
        in_maps.append(m)

    nc = _build_program()
    res = run_bass_kernel_spmd(
        nc, in_maps, list(range(N_CORES)),
        trace=bool(int(os.environ.get("KERNEL_TRACE", "0"))),
    )
    LAST_RESULT = res
    # out_d[p, t, b] = out[b, base + t*128 + p]
    out = np.empty((B, OUT_DIM), dtype=np.float32)
    for c in range(N_CORES):
        o = res.results[c]["out"].astype(np.float32)  # [128, NT, B]
        out[:, c * ROWS_PER_CORE:(c + 1) * ROWS_PER_CORE] = (
            o.transpose(1, 0, 2).reshape(ROWS_PER_CORE, B).T
        )
    return out


# revision 14
# speedup vs baseline: 1.1156x; 1.1156x over previous
"""Trainium2 kernel for nn_KeyedLayer: out = (W_sparse @ x.T).T

W is [16384, 16384] sparse COO (rows sorted, ~128 nnz/row, 2M nnz),
x is [64, 16384] fp32.  Strategy: shard output rows across 8 cores
(2048 rows each; disjoint outputs, no collectives).  Each core computes
out.T[2048, 64] = W_core @ x.T as a dense matmul with W densified on
the host in fp8 e3m4 (1.3e-2 rel err, well within tolerance) as the
stationary operand and x fp16 as the moving operand; K=16384 contracted
in 128 blocks through PSUM.

The kernel is DMA-bound: 32 MiB of fp8 W per core at ~360 B/ns (~93 us)
plus 2 MiB x.  W streams in two row-half passes (rows 0-1023 across all
k, then 1024-2047) so the first half's PSUM copyback and store are
hidden under the second pass; the final chunks are single k-blocks so
the tail after the last byte is short.
"""

import os
from contextlib import ExitStack

import numpy as np
import ml_dtypes

import concourse.bass as bass
import concourse.tile as tile
from concourse import bacc, mybir
from concourse.bass_utils import run_bass_kernel_spmd

B = 64
IN_DIM = 16384
OUT_DIM = 16384
N_CORES = 8
ROWS_PER_CORE = OUT_DIM // N_CORES  # 2048
KBLK = IN_DIM // 128  # 128 k-blocks of 128
NT = ROWS_PER_CORE // 128  # 16 row-tiles of 128 rows
NH = NT // 2  # row-tiles per half (8 = one PSUM bank)
RH = ROWS_PER_CORE // 2  # rows per half (1024)

# Each half-pass streams KBLK k-blocks of [128, 1024] fp8 (128 KiB each),
# grouped into DMA chunks.  Pass 0 is all big chunks; pass 1 ends with
# small (2-kblock, 256 KiB) chunks so the post-last-byte matmul burst is
# short.  Chunk transfers must stay >= the ~650 ns SP dispatch+HWDGE cost
# or the DMA engines starve, so tail chunks are 2 k-blocks, not 1.
KGRP = int(os.environ.get("KERNEL_KGRP", "16"))
TGRP = int(os.environ.get("KERNEL_TGRP", "2"))   # k-blocks per tail chunk
NTAIL = int(os.environ.get("KERNEL_NTAIL", "8"))  # tail chunks (pass 1)
# per-pass chunking: list of (kblocks_per_chunk, count)
PASS_CHUNKS = [
    [(KGRP, KBLK // KGRP)],
    [((KBLK - NTAIL * TGRP) // (KBLK // KGRP - 1), KBLK // KGRP - 1),
     (TGRP, NTAIL)],
]
for pc in PASS_CHUNKS:
    assert sum(g * n for g, n in pc) == KBLK, pc
WBUFS = int(os.environ.get("KERNEL_WBUFS", "6"))

F16 = mybir.dt.float16
FP8 = mybir.dt.float8e3  # e3m4
FP8_NP = ml_dtypes.float8_e3m4
F32 = mybir.dt.float32

_CACHE = {}

LAST_RESULT = None  # BassKernelResults of the most recent run (for test.py)


def _build_program():
    if "nc" in _CACHE:
        return _CACHE["nc"]
    nc = bacc.Bacc(
        "TRN2", target_bir_lowering=False, debug=False, num_devices=N_CORES
    )
    xT_d = nc.dram_tensor("xT", [128, KBLK, B], F16, kind="ExternalInput")
    # wt[h, k, p, r] = W[base + h*1024 + r, k*128 + p]
    wt_d = nc.dram_tensor("wt", [2, KBLK, 128, RH], FP8,
                          kind="ExternalInput")
    # out[p, t, b] = out[b, base + t*128 + p] (host untangles the layout)
    out_d = nc.dram_tensor("out", [128, NT, B], F16, kind="ExternalOutput")

    with tile.TileContext(nc) as tc, ExitStack() as ctx:
        xpool = ctx.enter_context(tc.tile_pool(name="x", bufs=1))
        wpool = ctx.enter_context(tc.tile_pool(name="w", bufs=WBUFS))
        opool = ctx.enter_context(tc.tile_pool(name="o", bufs=1))
        pspool = ctx.enter_context(
            tc.tile_pool(name="ps", bufs=1, space=bass.MemorySpace.PSUM)
        )

        xsb = xpool.tile([128, KBLK, B], F16)  # 2 MiB
        nc.sync.dma_start(xsb[:], xT_d[:])

        # out.T as 16 tiles of [128 rows, 64 batch] fp32 = 4 KiB/partition;
        # half h lives in its own PSUM bank.
        psum = pspool.tile([128, NT, B], F32)
        osb = opool.tile([128, NT, B], F16)

        for h in range(2):
            def kblock_matmuls(wap, k):
                # wap: [128 (k-part), NH*128 rows] fp8 for this k-block
                for i in range(NH):
                    t = h * NH + i
                    nc.tensor.matmul(
                        psum[:, t, :],
                        wap[:, i * 128:(i + 1) * 128],  # lhsT [128, 128] fp8
                        xsb[:, k, :],                   # rhs  [128, 64] fp16
                        # PSUM "start" zeroes the whole 2 KiB bank (zero
                        # region): exactly one start/stop per bank.
                        start=(k == 0 and i == 0),
                        stop=(k == KBLK - 1 and i == NH - 1),
                        skip_group_check=True,
                    )

            k0 = 0
            for grp, cnt in PASS_CHUNKS[h]:
                for _ in range(cnt):
                    wsb = wpool.tile([128, grp, RH], FP8)
                    nc.sync.dma_start(
                        wsb[:],
                        wt_d.ap()[h, k0:k0 + grp].rearrange("g p r -> p g r"),
                    )
                    for j in range(grp):
                        kblock_matmuls(wsb[:, j, :], k0 + j)
                    k0 += grp

            # PSUM -> SBUF fp16 copyback for this half on DVE, then store
            # via a DVE-issued DMA (skips the SP dispatch hop).  For h=0
            # this hides entirely under the h=1 W stream.
            sl = slice(h * NH, (h + 1) * NH)
            nc.vector.tensor_copy(osb[:, sl, :], psum[:, sl, :])
            nc.scalar.dma_start(out_d.ap()[:, sl, :], osb[:, sl, :])

    nc.compile()
    _CACHE["nc"] = nc
    return nc


def kernel(x_affine: np.ndarray, rows: np.ndarray, cols: np.ndarray,
           vals: np.ndarray) -> np.ndarray:
    global LAST_RESULT
    import scipy.sparse as sp

    x_affine = np.asarray(x_affine, dtype=np.float32)
    rows = np.asarray(rows, dtype=np.int64)
    cols = np.asarray(cols, dtype=np.int64)
    vals = np.asarray(vals, dtype=np.float32)

    # xT host layout [p, k, b]: element = x[b, k*128 + p]
    xT = np.ascontiguousarray(
        x_affine.T.reshape(KBLK, 128, B).transpose(1, 0, 2)
    ).astype(np.float16)

    # rows is sorted; slice each core's nnz range and densify only its
    # [16384, 2048] W.T block (duplicates are summed by scipy).
    in_maps = []
    for c in range(N_CORES):
        base = c * ROWS_PER_CORE
        lo, hi = np.searchsorted(rows, [base, base + ROWS_PER_CORE])
        w_slice = sp.coo_matrix(
            (vals[lo:hi], (cols[lo:hi], rows[lo:hi] - base)),
            shape=(IN_DIM, ROWS_PER_CORE),
        ).toarray()  # [16384, 2048] fp32, w_slice[k, r] = W[base+r, k]
        # wf[k, p, h, r] = W[base + h*1024 + r, k*128 + p] -> [h, k, p, r]
        wf = w_slice.astype(FP8_NP).reshape(KBLK, 128, 2, RH)
        wt = np.ascontiguousarray(wf.transpose(2, 0, 1, 3))
        in_maps.append({"xT": xT, "wt": wt})

    nc = _build_program()
    res = run_bass_kernel_spmd(
        nc, in_maps, list(range(N_CORES)),
        trace=bool(int(os.environ.get("KERNEL_TRACE", "0"))),
    )
    LAST_RESULT = res
    # out_d[p, t, b] = out[b, base + t*128 + p]
    out = np.empty((B, OUT_DIM), dtype=np.float32)
    for c in range(N_CORES):
        o = res.results[c]["out"].astype(np.float32)  # [128, NT, B]
        out[:, c * ROWS_PER_CORE:(c + 1) * ROWS_PER_CORE] = (
            o.transpose(1, 0, 2).reshape(ROWS_PER_CORE, B).T
        )
    return out


# revision 15
# speedup vs baseline: 1.1233x; 1.0069x over previous
"""Trainium2 kernel for nn_KeyedLayer: out = (W_sparse @ x.T).T

W is [16384, 16384] sparse COO (rows sorted, ~128 nnz/row, 2M nnz),
x is [64, 16384] fp32.  Strategy: shard output rows across 8 cores
(2048 rows each; disjoint outputs, no collectives).  Each core computes
out.T[2048, 64] = W_core @ x.T as a dense matmul with W densified on
the host in fp8 e3m4 (1.3e-2 rel err, well within tolerance) as the
stationary operand and x fp16 as the moving operand; K=16384 contracted
in 128 blocks through PSUM.

The kernel is DMA-bound: 32 MiB of fp8 W per core at ~360 B/ns (~93 us)
plus 2 MiB x.  W streams in two row-half passes (rows 0-1023 across all
k, then 1024-2047) so the first half's PSUM copyback and store are
hidden under the second pass; the final chunks are single k-blocks so
the tail after the last byte is short.
"""

import os
from contextlib import ExitStack

import numpy as np
import ml_dtypes

import concourse.bass as bass
import concourse.tile as tile
from concourse import bacc, mybir
from concourse.bass_utils import run_bass_kernel_spmd

B = 64
IN_DIM = 16384
OUT_DIM = 16384
N_CORES = 8
ROWS_PER_CORE = OUT_DIM // N_CORES  # 2048
KBLK = IN_DIM // 128  # 128 k-blocks of 128
NT = ROWS_PER_CORE // 128  # 16 row-tiles of 128 rows
NH = NT // 2  # row-tiles per half (8 = one PSUM bank)
RH = ROWS_PER_CORE // 2  # rows per half (1024)

# Each half-pass streams KBLK k-blocks of [128, 1024] fp8 (128 KiB each),
# grouped into DMA chunks.  Pass 0 is all big chunks; pass 1 ends with
# small (2-kblock, 256 KiB) chunks so the post-last-byte matmul burst is
# short.  Chunk transfers must stay >= the ~650 ns SP dispatch+HWDGE cost
# or the DMA engines starve, so tail chunks are 2 k-blocks, not 1.
KGRP = int(os.environ.get("KERNEL_KGRP", "16"))
TGRP = int(os.environ.get("KERNEL_TGRP", "2"))   # k-blocks per tail chunk
NTAIL = int(os.environ.get("KERNEL_NTAIL", "8"))  # tail chunks (pass 1)
# per-pass chunking: list of (kblocks_per_chunk, count)
PASS_CHUNKS = [
    [(KGRP, KBLK // KGRP)],
    [((KBLK - NTAIL * TGRP) // (KBLK // KGRP - 1), KBLK // KGRP - 1),
     (TGRP, NTAIL)],
]
for pc in PASS_CHUNKS:
    assert sum(g * n for g, n in pc) == KBLK, pc
WBUFS = int(os.environ.get("KERNEL_WBUFS", "9"))

F16 = mybir.dt.float16
FP8 = mybir.dt.float8e3  # e3m4
FP8_NP = ml_dtypes.float8_e3m4
F32 = mybir.dt.float32

_CACHE = {}

LAST_RESULT = None  # BassKernelResults of the most recent run (for test.py)


def _build_program():
    if "nc" in _CACHE:
        return _CACHE["nc"]
    nc = bacc.Bacc(
        "TRN2", target_bir_lowering=False, debug=False, num_devices=N_CORES
    )
    xT_d = nc.dram_tensor("xT", [128, KBLK, B], F16, kind="ExternalInput")
    # wt[h, k, p, r] = W[base + h*1024 + r, k*128 + p]
    wt_d = nc.dram_tensor("wt", [2, KBLK, 128, RH], FP8,
                          kind="ExternalInput")
    # out[p, t, b] = out[b, base + t*128 + p] (host untangles the layout)
    out_d = nc.dram_tensor("out", [128, NT, B], F16, kind="ExternalOutput")

    with tile.TileContext(nc) as tc, ExitStack() as ctx:
        xpool = ctx.enter_context(tc.tile_pool(name="x", bufs=1))
        wpool = ctx.enter_context(tc.tile_pool(name="w", bufs=WBUFS))
        tpool = ctx.enter_context(tc.tile_pool(name="wt", bufs=NTAIL))
        opool = ctx.enter_context(tc.tile_pool(name="o", bufs=1))
        pspool = ctx.enter_context(
            tc.tile_pool(name="ps", bufs=1, space=bass.MemorySpace.PSUM)
        )

        xsb = xpool.tile([128, KBLK, B], F16)  # 2 MiB
        nc.sync.dma_start(xsb[:], xT_d[:])

        # out.T as 16 tiles of [128 rows, 64 batch] fp32 = 4 KiB/partition;
        # half h lives in its own PSUM bank.
        psum = pspool.tile([128, NT, B], F32)
        osb = opool.tile([128, NT, B], F16)

        for h in range(2):
            def kblock_matmuls(wap, k):
                # wap: [128 (k-part), NH*128 rows] fp8 for this k-block
                for i in range(NH):
                    t = h * NH + i
                    nc.tensor.matmul(
                        psum[:, t, :],
                        wap[:, i * 128:(i + 1) * 128],  # lhsT [128, 128] fp8
                        xsb[:, k, :],                   # rhs  [128, 64] fp16
                        # PSUM "start" zeroes the whole 2 KiB bank (zero
                        # region): exactly one start/stop per bank.
                        start=(k == 0 and i == 0),
                        stop=(k == KBLK - 1 and i == NH - 1),
                        skip_group_check=True,
                    )

            k0 = 0
            for grp, cnt in PASS_CHUNKS[h]:
                for _ in range(cnt):
                    pool = wpool if grp >= KGRP else tpool
                    wsb = pool.tile([128, grp, RH], FP8)
                    nc.sync.dma_start(
                        wsb[:],
                        wt_d.ap()[h, k0:k0 + grp].rearrange("g p r -> p g r"),
                    )
                    for j in range(grp):
                        kblock_matmuls(wsb[:, j, :], k0 + j)
                    k0 += grp

            # PSUM -> SBUF fp16 copyback for this half on DVE, then store
            # via a DVE-issued DMA (skips the SP dispatch hop).  For h=0
            # this hides entirely under the h=1 W stream.
            sl = slice(h * NH, (h + 1) * NH)
            nc.vector.tensor_copy(osb[:, sl, :], psum[:, sl, :])
            nc.scalar.dma_start(out_d.ap()[:, sl, :], osb[:, sl, :])

    nc.compile()
    _CACHE["nc"] = nc
    return nc


def kernel(x_affine: np.ndarray, rows: np.ndarray, cols: np.ndarray,
           vals: np.ndarray) -> np.ndarray:
    global LAST_RESULT
    import scipy.sparse as sp

    x_affine = np.asarray(x_affine, dtype=np.float32)
    rows = np.asarray(rows, dtype=np.int64)
    cols = np.asarray(cols, dtype=np.int64)
    vals = np.asarray(vals, dtype=np.float32)

    # xT host layout [p, k, b]: element = x[b, k*128 + p]
    xT = np.ascontiguousarray(
        x_affine.T.reshape(KBLK, 128, B).transpose(1, 0, 2)
    ).astype(np.float16)

    # rows is sorted; slice each core's nnz range and densify only its
    # [16384, 2048] W.T block (duplicates are summed by scipy).
    in_maps = []
    for c in range(N_CORES):
        base = c * ROWS_PER_CORE
        lo, hi = np.searchsorted(rows, [base, base + ROWS_PER_CORE])
        w_slice = sp.coo_matrix(
            (vals[lo:hi], (cols[lo:hi], rows[lo:hi] - base)),
            shape=(IN_DIM, ROWS_PER_CORE),
        ).toarray()  # [16384, 2048] fp32, w_slice[k, r] = W[base+r, k]
        # wf[k, p, h, r] = W[base + h*1024 + r, k*128 + p] -> [h, k, p, r]
        wf = w_slice.astype(FP8_NP).reshape(KBLK, 128, 2, RH)
        wt = np.ascontiguousarray(wf.transpose(2, 0, 1, 3))
        in_maps.append({"xT": xT, "wt": wt})

    nc = _build_program()
    res = run_bass_kernel_spmd(
        nc, in_maps, list(range(N_CORES)),
        trace=bool(int(os.environ.get("KERNEL_TRACE", "0"))),
    )
    LAST_RESULT = res
    # out_d[p, t, b] = out[b, base + t*128 + p]
    out = np.empty((B, OUT_DIM), dtype=np.float32)
    for c in range(N_CORES):
        o = res.results[c]["out"].astype(np.float32)  # [128, NT, B]
        out[:, c * ROWS_PER_CORE:(c + 1) * ROWS_PER_CORE] = (
            o.transpose(1, 0, 2).reshape(ROWS_PER_CORE, B).T
        )
    return out
